# revision 17
# baseline (speedup 1.0000x reference)
"""Trainium2 Bass kernel for AdaptiveGraphLearning (retrieval_knn).

Computes, for X [8192,128], A_raw [8192,8192], lambda scalar:
  Xn = X / max(||X||_2, 1e-12)   (row-normalize)
  S  = Xn @ Xn.T                 (cosine similarity)
  A  = dense top-(K+1) per row with self-edge dropped, row-normalized
  A_final = sigmoid(lam)*A_raw + (1-sigmoid(lam))*A_learned
Returns (A_final, A_learned).

Distribution: row-shard N across 8 cores (1024 rows/core). Each core gets
the full X but ROTATED by its row offset, so in its local coordinates its
rows are 0..1024 and the self-similarity diagonal of row-tile t always
falls at local columns [t*128,(t+1)*128) -- the SPMD graph is identical on
all cores. A_raw shards are column-rotated the same way on the host, and
the outputs are un-rotated after the gather.

Top-k without indices: per row, the 10th-largest off-diagonal similarity
(tau) comes from per-1024-chunk top-8 (DVE max8) -> 64 candidates ->
top-16 via max8 + match_replace + max8. Selection is one fused pass:
SEL = (S >= tau) * S; the row sum is the sum of the top-10 values taken
from the max8 outputs directly.
"""

import numpy as np

N = 8192
D = 128
NCORES = 8
RPC = N // NCORES   # rows per core
P = 128
TILES = RPC // P    # row tiles per core
MMF = 512           # matmul moving free dim (one PSUM bank, f32)
CH = 1024           # max8 chunk width (two PSUM banks)
NCH = N // CH       # chunks per row: 8
CAND = NCH * 8      # candidates per row: 64
XG = 8              # X prologue groups
XT_PER_G = (N // P) // XG  # x row-tiles per group: 8
EPQ = 2048          # epilogue column chunk
NEP = N // EPQ      # epilogue chunks: 4

import os
USE_F32R = os.environ.get("USE_F32R", "0") == "1"

LAST_RESULTS = None
_NC_CACHE = None


def _build():
    import concourse.mybir as mybir
    import concourse.tile as tile
    from concourse import bacc
    from concourse.bass import ts
    from concourse.masks import make_identity

    f32 = mybir.dt.float32
    AF = mybir.ActivationFunctionType
    OP = mybir.AluOpType

    nc = bacc.Bacc("TRN2", target_bir_lowering=False, debug=False,
                   num_devices=NCORES)

    X_d = nc.dram_tensor("X", [N, D], f32, kind="ExternalInput")
    A_d = nc.dram_tensor("A_raw", [RPC, N], f32, kind="ExternalInput")
    lam_d = nc.dram_tensor("lam", [P, 1], f32, kind="ExternalInput")
    AF_d = nc.dram_tensor("A_final", [RPC, N], f32, kind="ExternalOutput")
    AL_d = nc.dram_tensor("A_learned", [RPC, N], f32, kind="ExternalOutput")

    with tile.TileContext(nc) as tc:
        with (
            tc.tile_pool(name="const", bufs=1) as constp,
            tc.tile_pool(name="xnt", bufs=1) as xntp,
            tc.tile_pool(name="selp", bufs=2) as selp,
            tc.tile_pool(name="arawp", bufs=2) as arawp,
            tc.tile_pool(name="small", bufs=2) as smallp,
            tc.tile_pool(name="psum", bufs=4, space="PSUM") as psump,
        ):
            # lambda: sigmoid on device; host replicates the scalar to [128,1]
            lam_sb = constp.tile([P, 1], f32, name="lam_sb")
            nc.sync.dma_start(lam_sb[:], lam_d.ap())
            lam_bc = constp.tile([P, 1], f32, name="lam_bc")
            nc.scalar.activation(lam_bc[:], lam_sb[:], AF.Sigmoid)
            omlam = constp.tile([P, 1], f32, name="omlam")
            nc.scalar.activation(omlam[:], lam_bc[:], AF.Copy, bias=1.0,
                                 scale=-1.0)

            ident = constp.tile([P, P], f32, name="ident")
            make_identity(nc, ident[:])
            # notI: 1 everywhere except 0 on the diagonal
            notI = constp.tile([P, P], f32, name="notI")
            nc.gpsimd.memset(notI[:], 1.0)
            nc.gpsimd.affine_select(
                out=notI[:], in_=notI[:], pattern=[[-1, P]],
                compare_op=OP.not_equal, fill=0.0, base=0,
                channel_multiplier=1)

            # A_raw DMA prefetch can run during the prologue (independent of X)
            araw_tiles = {}
            def fetch_araw(t):
                araw_t = arawp.tile([P, N], f32, name=f"araw{t}", tag="araw")
                nc.sync.dma_start(araw_t[:], A_d.ap()[ts(t, P), :])
                araw_tiles[t] = araw_t

            fetch_araw(0)

            # X prologue, pipelined in XG groups: load [p, tt, d] slices,
            # row-normalize, PE-transpose into XnT [D, N].
            xnt_dt = mybir.dt.float32r if USE_F32R else f32
            xnt = xntp.tile([P, N], xnt_dt, name="xnt")
            xt = selp.tile([P, N // P, D], f32, name="xt", tag="sel")
            sq = selp.tile([P, N // P, D], f32, name="sq", tag="sel")
            n2 = constp.tile([P, N // P], f32, name="n2")
            invn = constp.tile([P, N // P], f32, name="invn")
            xr = X_d.ap().rearrange("(t p) d -> p t d", p=P)
            for g in range(XG):
                gsl = ts(g, XT_PER_G)
                nc.sync.dma_start(xt[:, gsl, :], xr[:, gsl, :])
                nc.scalar.activation(sq[:, gsl, :], xt[:, gsl, :], AF.Square)
                nc.vector.reduce_sum(n2[:, gsl], sq[:, gsl, :],
                                     axis=mybir.AxisListType.X)
                nc.scalar.activation(invn[:, gsl], n2[:, gsl], AF.Sqrt)
                nc.vector.tensor_scalar_max(invn[:, gsl], invn[:, gsl], 1e-12)
                nc.vector.reciprocal(invn[:, gsl], invn[:, gsl])
                nc.vector.tensor_mul(
                    xt[:, gsl, :], xt[:, gsl, :],
                    invn[:, gsl, None].to_broadcast((P, XT_PER_G, D)))
                # two PE transposes per PSUM tile -> one xnt copy of 256 cols
                for tt in range(g * XT_PER_G, (g + 1) * XT_PER_G, 2):
                    pt = psump.tile([P, CH], f32, name=f"tp{tt}", tag="mm")
                    nc.tensor.transpose(pt[:, 0:P], xt[:, tt, :], ident[:])
                    nc.tensor.transpose(pt[:, P:2 * P], xt[:, tt + 1, :],
                                        ident[:])
                    nc.scalar.copy(xnt[:, tt * P:(tt + 2) * P], pt[:, 0:2 * P])

            fetch_araw(1)

            for t in range(TILES):
                araw_t = araw_tiles.pop(t)
                # AR = lam*A_raw (ACT, in place), independent of the S pipeline
                for q in range(NEP):
                    nc.scalar.activation(araw_t[:, ts(q, EPQ)],
                                         araw_t[:, ts(q, EPQ)], AF.Copy,
                                         scale=lam_bc[:])
                s_t = selp.tile([P, N], f32, name=f"s{t}", tag="sel")
                cand = smallp.tile([P, CAND], f32, name=f"cand{t}", tag="cand")
                diag_chunk = (t * P) // CH
                for c in range(NCH):
                    pm = psump.tile([P, CH], f32, name=f"mm{t}_{c}", tag="mm")
                    nc.tensor.matmul(pm[:, 0:MMF], xnt[:, ts(t, P)],
                                     xnt[:, ts(2 * c, MMF)],
                                     start=True, stop=True)
                    nc.tensor.matmul(pm[:, MMF:CH], xnt[:, ts(t, P)],
                                     xnt[:, ts(2 * c + 1, MMF)],
                                     start=True, stop=True)
                    nc.scalar.copy(s_t[:, ts(c, CH)], pm[:])
                    if c == diag_chunk:
                        # zero the self-similarity diagonal
                        nc.vector.tensor_mul(s_t[:, ts(t, P)],
                                             s_t[:, ts(t, P)], notI[:])
                    nc.vector.max(cand[:, ts(c, 8)], s_t[:, ts(c, CH)])

                # g12 holds top-8 (g1) then 9th..16th (g2) adjacently, so the
                # top-10 row sum is one reduce over g12[:, 0:10]
                g12 = smallp.tile([P, 16], f32, name=f"g12_{t}", tag="g12")
                nc.vector.max(g12[:, 0:8], cand[:])
                nc.vector.match_replace(out=cand[:], in_to_replace=g12[:, 0:8],
                                        in_values=cand[:], imm_value=-1e30)
                nc.vector.max(g12[:, 8:16], cand[:])

                den = smallp.tile([P, 1], f32, name=f"den{t}", tag="den")
                nc.vector.reduce_sum(den[:], g12[:, 0:10],
                                     axis=mybir.AxisListType.X)
                nc.vector.tensor_scalar_add(den[:], den[:], 1e-6)
                invr = smallp.tile([P, 1], f32, name=f"invr{t}", tag="invr")
                nc.vector.reciprocal(invr[:], den[:])

                # SEL = (S >= tau) * S, in place on s_t; tau = g12[:,9]
                nc.vector.scalar_tensor_tensor(
                    out=s_t[:], in0=s_t[:], scalar=g12[:, 9:10], in1=s_t[:],
                    op0=OP.is_ge, op1=OP.mult)

                if t + 2 < TILES:
                    fetch_araw(t + 2)

                # chunked epilogue: A_learned = SEL*invr (ACT, in place);
                # A_final = (1-lam)*A_learned + lam*A_raw (DVE, in place on
                # araw_t, which already holds lam*A_raw); outputs DMA per chunk
                for q in range(NEP):
                    qs = ts(q, EPQ)
                    nc.scalar.activation(s_t[:, qs], s_t[:, qs], AF.Copy,
                                         scale=invr[:])
                    nc.vector.scalar_tensor_tensor(
                        out=araw_t[:, qs], in0=s_t[:, qs], scalar=omlam[:],
                        in1=araw_t[:, qs], op0=OP.mult, op1=OP.add)
                    nc.sync.dma_start(AL_d.ap()[ts(t, P), qs], s_t[:, qs])
                    nc.sync.dma_start(AF_d.ap()[ts(t, P), qs], araw_t[:, qs])

    nc.compile()
    return nc


def kernel(X, A_raw, lambda_param):
    global LAST_RESULTS, _NC_CACHE
    from concourse.bass_utils import run_bass_kernel_spmd

    X = np.ascontiguousarray(np.asarray(X, dtype=np.float32))
    A_raw = np.asarray(A_raw, dtype=np.float32)
    lam = float(np.asarray(lambda_param, dtype=np.float32).reshape(()))

    if _NC_CACHE is None:
        _NC_CACHE = _build()
    nc = _NC_CACHE

    lam_in = np.full((P, 1), lam, dtype=np.float32)
    in_maps = []
    for c in range(NCORES):
        r0 = c * RPC
        in_maps.append({
            "X": np.roll(X, -r0, axis=0),
            "A_raw": np.ascontiguousarray(np.roll(A_raw[r0:r0 + RPC], -r0,
                                                  axis=1)),
            "lam": lam_in,
        })

    res = run_bass_kernel_spmd(nc, in_maps, core_ids=list(range(NCORES)))
    LAST_RESULTS = res

    A_final = np.empty((N, N), dtype=np.float32)
    A_learned = np.empty((N, N), dtype=np.float32)
    for c in range(NCORES):
        r0 = c * RPC
        A_final[r0:r0 + RPC] = np.roll(res.results[c]["A_final"], r0, axis=1)
        A_learned[r0:r0 + RPC] = np.roll(res.results[c]["A_learned"], r0,
                                         axis=1)
    return A_final, A_learned


# revision 18
# speedup vs baseline: 1.1007x; 1.1007x over previous
"""Trainium2 Bass kernel for AdaptiveGraphLearning (retrieval_knn).

Computes, for X [8192,128], A_raw [8192,8192], lambda scalar:
  Xn = X / max(||X||_2, 1e-12)   (row-normalize)
  S  = Xn @ Xn.T                 (cosine similarity)
  A  = dense top-(K+1) per row with self-edge dropped, row-normalized
  A_final = sigmoid(lam)*A_raw + (1-sigmoid(lam))*A_learned
Returns (A_final, A_learned).

Distribution: row-shard N across 8 cores (1024 rows/core). Each core gets
the full X but ROTATED by its row offset, so in its local coordinates its
rows are 0..1024 and the self-similarity diagonal of row-tile t always
falls at local columns [t*128,(t+1)*128) -- the SPMD graph is identical on
all cores. A_raw shards are column-rotated the same way on the host, and
the outputs are un-rotated after the gather.

Top-k without indices: per row, the 10th-largest off-diagonal similarity
(tau) comes from per-1024-chunk top-8 (DVE max8) -> 64 candidates ->
top-16 via max8 + match_replace + max8. Selection is one fused pass:
SEL = (S >= tau) * S; the row sum is the sum of the top-10 values taken
from the max8 outputs directly.
"""

import numpy as np

N = 8192
D = 128
NCORES = 8
RPC = N // NCORES   # rows per core
P = 128
TILES = RPC // P    # row tiles per core
MMF = 512           # matmul moving free dim (one PSUM bank, f32)
CH = 1024           # max8 chunk width (two PSUM banks)
NCH = N // CH       # chunks per row: 8
CAND = NCH * 8      # candidates per row: 64
XG = 8              # X prologue groups
XT_PER_G = (N // P) // XG  # x row-tiles per group: 8
EPQ = 2048          # epilogue column chunk
NEP = N // EPQ      # epilogue chunks: 4

import os
USE_F32R = os.environ.get("USE_F32R", "0") == "1"

LAST_RESULTS = None
_NC_CACHE = None


def _build():
    import concourse.mybir as mybir
    import concourse.tile as tile
    from concourse import bacc
    from concourse.bass import ts
    from concourse.masks import make_identity

    f32 = mybir.dt.float32
    AF = mybir.ActivationFunctionType
    OP = mybir.AluOpType

    nc = bacc.Bacc("TRN2", target_bir_lowering=False, debug=False,
                   num_devices=NCORES)

    X_d = nc.dram_tensor("X", [N, D], f32, kind="ExternalInput")
    A_d = nc.dram_tensor("A_raw", [RPC, N], f32, kind="ExternalInput")
    lam_d = nc.dram_tensor("lam", [P, 1], f32, kind="ExternalInput")
    AF_d = nc.dram_tensor("A_final", [RPC, N], f32, kind="ExternalOutput")
    AL_d = nc.dram_tensor("A_learned", [RPC, N], f32, kind="ExternalOutput")

    with tile.TileContext(nc) as tc:
        with (
            tc.tile_pool(name="const", bufs=1) as constp,
            tc.tile_pool(name="xnt", bufs=1) as xntp,
            tc.tile_pool(name="selp", bufs=2) as selp,
            tc.tile_pool(name="arawp", bufs=2) as arawp,
            tc.tile_pool(name="small", bufs=2) as smallp,
            tc.tile_pool(name="psum", bufs=4, space="PSUM") as psump,
        ):
            # lambda: sigmoid on device; host replicates the scalar to [128,1]
            lam_sb = constp.tile([P, 1], f32, name="lam_sb")
            nc.sync.dma_start(lam_sb[:], lam_d.ap())
            lam_bc = constp.tile([P, 1], f32, name="lam_bc")
            nc.scalar.activation(lam_bc[:], lam_sb[:], AF.Sigmoid)
            omlam = constp.tile([P, 1], f32, name="omlam")
            nc.scalar.activation(omlam[:], lam_bc[:], AF.Copy, bias=1.0,
                                 scale=-1.0)

            ident = constp.tile([P, P], f32, name="ident")
            make_identity(nc, ident[:])
            # notI: 1 everywhere except 0 on the diagonal
            notI = constp.tile([P, P], f32, name="notI")
            nc.gpsimd.memset(notI[:], 1.0)
            nc.gpsimd.affine_select(
                out=notI[:], in_=notI[:], pattern=[[-1, P]],
                compare_op=OP.not_equal, fill=0.0, base=0,
                channel_multiplier=1)

            # A_raw DMA prefetch can run during the prologue (independent of X)
            araw_tiles = {}
            def fetch_araw(t):
                araw_t = arawp.tile([P, N], f32, name=f"araw{t}", tag="araw")
                nc.sync.dma_start(araw_t[:], A_d.ap()[ts(t, P), :])
                araw_tiles[t] = araw_t

            fetch_araw(0)

            # X prologue, pipelined in XG groups: load [p, tt, d] slices,
            # row-normalize, PE-transpose into XnT [D, N].
            xnt_dt = mybir.dt.float32r if USE_F32R else f32
            xnt = xntp.tile([P, N], xnt_dt, name="xnt")
            xt = selp.tile([P, N // P, D], f32, name="xt", tag="sel")
            sq = selp.tile([P, N // P, D], f32, name="sq", tag="sel")
            n2 = constp.tile([P, N // P], f32, name="n2")
            invn = constp.tile([P, N // P], f32, name="invn")
            xr = X_d.ap().rearrange("(t p) d -> p t d", p=P)
            for g in range(XG):
                gsl = ts(g, XT_PER_G)
                nc.sync.dma_start(xt[:, gsl, :], xr[:, gsl, :])
                nc.scalar.activation(sq[:, gsl, :], xt[:, gsl, :], AF.Square)
                nc.vector.reduce_sum(n2[:, gsl], sq[:, gsl, :],
                                     axis=mybir.AxisListType.X)
                nc.scalar.activation(invn[:, gsl], n2[:, gsl], AF.Sqrt)
                nc.vector.tensor_scalar_max(invn[:, gsl], invn[:, gsl], 1e-12)
                nc.vector.reciprocal(invn[:, gsl], invn[:, gsl])
                nc.vector.tensor_mul(
                    xt[:, gsl, :], xt[:, gsl, :],
                    invn[:, gsl, None].to_broadcast((P, XT_PER_G, D)))
                for tt in range(g * XT_PER_G, (g + 1) * XT_PER_G):
                    pt = psump.tile([P, CH], f32, name=f"tp{tt}", tag="mm")
                    nc.tensor.transpose(pt[:, :P], xt[:, tt, :], ident[:])
                    nc.scalar.copy(xnt[:, ts(tt, P)], pt[:, :P])

            fetch_araw(1)

            for t in range(TILES):
                araw_t = araw_tiles.pop(t)
                # AR = lam*A_raw (ACT, in place), independent of the S pipeline
                for q in range(NEP):
                    nc.scalar.activation(araw_t[:, ts(q, EPQ)],
                                         araw_t[:, ts(q, EPQ)], AF.Copy,
                                         scale=lam_bc[:])
                s_t = selp.tile([P, N], f32, name=f"s{t}", tag="sel")
                cand = smallp.tile([P, CAND], f32, name=f"cand{t}", tag="cand")
                diag_chunk = (t * P) // CH
                for c in range(NCH):
                    pm = psump.tile([P, CH], f32, name=f"mm{t}_{c}", tag="mm")
                    nc.tensor.matmul(pm[:, 0:MMF], xnt[:, ts(t, P)],
                                     xnt[:, ts(2 * c, MMF)],
                                     start=True, stop=True)
                    nc.tensor.matmul(pm[:, MMF:CH], xnt[:, ts(t, P)],
                                     xnt[:, ts(2 * c + 1, MMF)],
                                     start=True, stop=True)
                    nc.scalar.copy(s_t[:, ts(c, CH)], pm[:])
                    if c == diag_chunk:
                        # zero the self-similarity diagonal
                        nc.vector.tensor_mul(s_t[:, ts(t, P)],
                                             s_t[:, ts(t, P)], notI[:])
                    nc.vector.max(cand[:, ts(c, 8)], s_t[:, ts(c, CH)])

                g1 = smallp.tile([P, 8], f32, name=f"g1_{t}", tag="g1")
                nc.vector.max(g1[:], cand[:])
                nc.vector.match_replace(out=cand[:], in_to_replace=g1[:],
                                        in_values=cand[:], imm_value=-1e30)
                g2 = smallp.tile([P, 8], f32, name=f"g2_{t}", tag="g2")
                nc.vector.max(g2[:], cand[:])

                rs1 = smallp.tile([P, 1], f32, name=f"rs1_{t}", tag="rs1")
                nc.vector.reduce_sum(rs1[:], g1[:], axis=mybir.AxisListType.X)
                rs2 = smallp.tile([P, 1], f32, name=f"rs2_{t}", tag="rs2")
                nc.vector.reduce_sum(rs2[:], g2[:, 0:2],
                                     axis=mybir.AxisListType.X)
                den = smallp.tile([P, 1], f32, name=f"den{t}", tag="den")
                nc.vector.tensor_add(den[:], rs1[:], rs2[:])
                nc.vector.tensor_scalar_add(den[:], den[:], 1e-6)
                invr = smallp.tile([P, 1], f32, name=f"invr{t}", tag="invr")
                nc.vector.reciprocal(invr[:], den[:])

                # SEL = (S >= tau) * S, in place on s_t; tau = g2[:,1]
                nc.vector.scalar_tensor_tensor(
                    out=s_t[:], in0=s_t[:], scalar=g2[:, 1:2], in1=s_t[:],
                    op0=OP.is_ge, op1=OP.mult)

                if t + 2 < TILES:
                    fetch_araw(t + 2)

                # chunked epilogue: A_learned = SEL*invr (ACT, in place);
                # A_final = (1-lam)*A_learned + lam*A_raw (DVE, in place on
                # araw_t, which already holds lam*A_raw); outputs DMA per chunk
                for q in range(NEP):
                    qs = ts(q, EPQ)
                    nc.scalar.activation(s_t[:, qs], s_t[:, qs], AF.Copy,
                                         scale=invr[:])
                    nc.vector.scalar_tensor_tensor(
                        out=araw_t[:, qs], in0=s_t[:, qs], scalar=omlam[:],
                        in1=araw_t[:, qs], op0=OP.mult, op1=OP.add)
                    nc.sync.dma_start(AL_d.ap()[ts(t, P), qs], s_t[:, qs])
                    nc.sync.dma_start(AF_d.ap()[ts(t, P), qs], araw_t[:, qs])

    nc.compile()
    return nc


def kernel(X, A_raw, lambda_param):
    global LAST_RESULTS, _NC_CACHE
    from concourse.bass_utils import run_bass_kernel_spmd

    X = np.ascontiguousarray(np.asarray(X, dtype=np.float32))
    A_raw = np.asarray(A_raw, dtype=np.float32)
    lam = float(np.asarray(lambda_param, dtype=np.float32).reshape(()))

    if _NC_CACHE is None:
        _NC_CACHE = _build()
    nc = _NC_CACHE

    lam_in = np.full((P, 1), lam, dtype=np.float32)
    in_maps = []
    for c in range(NCORES):
        r0 = c * RPC
        in_maps.append({
            "X": np.roll(X, -r0, axis=0),
            "A_raw": np.ascontiguousarray(np.roll(A_raw[r0:r0 + RPC], -r0,
                                                  axis=1)),
            "lam": lam_in,
        })

    res = run_bass_kernel_spmd(nc, in_maps, core_ids=list(range(NCORES)))
    LAST_RESULTS = res

    A_final = np.empty((N, N), dtype=np.float32)
    A_learned = np.empty((N, N), dtype=np.float32)
    for c in range(NCORES):
        r0 = c * RPC
        A_final[r0:r0 + RPC] = np.roll(res.results[c]["A_final"], r0, axis=1)
        A_learned[r0:r0 + RPC] = np.roll(res.results[c]["A_learned"], r0,
                                         axis=1)
    return A_final, A_learned
